# revision 32
# baseline (speedup 1.0000x reference)
"""Trainium2 Bass kernel for nn_CPDist.

Math: with a = exp(h_last @ W.T + b).reshape(B, H, V, R), the reference
computes p_tilde[b,i,j] = sum_r a[b,0,i,r]*a[b,1,j,r], then
  p_eval[b]     = p_tilde[b, p0, p1]
  norm_const[b] = sum_ij p_tilde[b,i,j]
Both factorize over the rank dim, so the (B,V,V) slab is never needed:
  norm_const[b] = sum_r (sum_i a[b,0,i,r]) * (sum_j a[b,1,j,r])
  p_eval[b]     = sum_r a[b,0,p0,r] * a[b,1,p1,r]
The dominant cost is the (B=8, D=1024) x (D, V*R*H=131072) matmul + exp —
HBM-bound on streaming the fp8 weight slab (16.8 MB per core).

Sharding: vocab dim V split across 8 cores (512 vocab rows each, for both
horizon slots).

Orientation: weights are the STATIONARY matmul operand (lhsT tiles of
[K=128, M=128 vocab]) and the hidden state h^T is the moving operand
([K=128, N=8]), so each matmul produces a [128 vocab, 8 batch] psum tile.
exp runs on [128, up to 128] tiles (full partition usage), vocab sums are
one ones-matmul per group (partition-dim reduction on the PE), and the
p_eval gather is one mask-matmul per group against host-built one-hot
columns. The last (1-chunk) group's exp tile is exported raw and reduced
on the host so the post-DMA tail stays minimal.
"""

import os

import numpy as np

import concourse.bacc as bacc
import concourse.bass as bass
import concourse.mybir as mybir
import concourse.tile as tile

B, T, D = 8, 128, 1024
V, R, H = 4096, 16, 2
NCORES = 8
VSH = V // NCORES            # vocab rows per core (512)
KT = D // 128                # 8 contraction tiles
NCHUNK = H * R               # 32 chunks of 512 vocab columns per core
NBLK = VSH // 128            # 4 vocab subtiles of 128 per chunk
CCOLS = NBLK * KT * 128      # 4096 weight-slab columns per chunk

F32 = mybir.dt.float32
BF16 = mybir.dt.bfloat16

MM_DTYPE = mybir.dt.float8e4
MM_SCALE = 1024.0

# chunks per weight DMA group; the final groups are small so the post-DMA
# drain only has one chunk of compute left
PLAN = [int(x) for x in os.environ.get("CPDIST_PLAN", "4,4,4,4,4,4,4,3,1").split(",")]
assert sum(PLAN) == NCHUNK
NGRP = len(PLAN)
GSTART = [sum(PLAN[:i]) for i in range(NGRP)]
NFULL = NGRP - 1             # groups that compute strip sums on device
LAST_CPD = PLAN[-1]

WBUFS = int(os.environ.get("CPDIST_WBUFS", "4"))
PSBUFS = int(os.environ.get("CPDIST_PSBUFS", "4"))
# debug: use plain HWDGE dma_start for the exports instead of triggered
# kv_writeback
PLAIN_OUT = os.environ.get("CPDIST_PLAIN_OUT", "0") == "1"

# strip column layout: cols [0, NFULL) = per-group vocab sums,
# cols [NFULL, NFULL + 8*NFULL) = per-group gather outputs
SGATHER = NFULL
STRIP_COLS = NFULL + B * NFULL

_cached = {}
_fast = {}
_last_results = None


def _to_mm(x, scale=1.0):
    x = np.ascontiguousarray(x, dtype=np.float32)
    if scale != 1.0:
        x = x * np.float32(scale)
    return x.astype(mybir.dt.np(MM_DTYPE))


def _build_nc(use_bias=False):
    nc = bacc.Bacc("TRN2", target_bir_lowering=False, num_swdge_queues=2)
    wt = nc.dram_tensor("wt", (128, NCHUNK * CCOLS), MM_DTYPE, kind="ExternalInput")
    pack2 = nc.dram_tensor("pack2", (128, KT * B), MM_DTYPE, kind="ExternalInput")
    # col 0: ones column for the vocab-sum matmuls; cols 1..1+8*NFULL:
    # one-hot gather masks (8 batch points per full group)
    cmask = nc.dram_tensor("cmask", (128, 1 + B * NFULL), BF16, kind="ExternalInput")
    if use_bias:
        bias_m = nc.dram_tensor("bias_m", (2, NCHUNK * VSH), F32, kind="ExternalInput")
    LCOLS = LAST_CPD * NBLK * B
    # exports are kv_writeback-shaped: [batch=1, d_head_inner=128, d_head_outer=1, n]
    strip_out = nc.dram_tensor("strip_out", (1, 128, 1, STRIP_COLS), BF16,
                               kind="ExternalOutput")
    l8_out = nc.dram_tensor("l8_out", (1, 128, 1, LCOLS), BF16, kind="ExternalOutput")

    with tile.TileContext(nc) as tc:
        with (
            tc.tile_pool(name="consts", bufs=1) as consts,
            tc.tile_pool(name="wpool", bufs=WBUFS) as wpool,
            tc.tile_pool(name="pspool", bufs=PSBUFS, space="PSUM") as pspool,
            tc.tile_pool(name="strip_pool", bufs=1, space="PSUM") as strip_pool,
            tc.tile_pool(name="epool", bufs=3) as epool,
            tc.tile_pool(name="spool", bufs=2) as spool,
        ):
            # descriptors for the two triggered output writebacks are
            # prepared up-front on the Pool engine; only the cheap
            # trigger_dma sits on the end-of-kernel critical path
            ctx_sb = consts.tile([128, 1], mybir.dt.int32, name="ctx_idx")
            nc.gpsimd.memset(ctx_sb[:], 0)
            strip_sb = spool.tile([128, STRIP_COLS], BF16, name="strip_sb")
            l8_sb = spool.tile([128, LCOLS], BF16, name="l8_sb")
            if not PLAIN_OUT:
                sem_strip = nc.alloc_semaphore("swdge_strip")
                sem_l8 = nc.alloc_semaphore("swdge_l8")
                sem_gate = nc.alloc_semaphore("out_gate")

            def issue_group(g):
                cpd = PLAN[g]
                w_tile = wpool.tile([128, cpd * CCOLS], MM_DTYPE,
                                    padded_shape=[128, max(PLAN) * CCOLS],
                                    name=f"w_tile_{g}", tag="w_tile")
                nc.sync.dma_start(
                    out=w_tile[:],
                    in_=wt[:, GSTART[g] * CCOLS:(GSTART[g] + cpd) * CCOLS],
                )
                return w_tile

            # first weight group goes ahead of everything so the DMA engines
            # start on the bulk stream as early as possible
            pre = [issue_group(0)]
            p2_sb = consts.tile([128, KT * B], MM_DTYPE)
            nc.sync.dma_start(out=p2_sb[:], in_=pack2[:])
            cm_sb = consts.tile([128, 1 + B * NFULL], BF16)
            nc.sync.dma_start(out=cm_sb[:], in_=cmask[:])
            if use_bias:
                bias_sb = consts.tile([2, NCHUNK * VSH], F32)
                nc.sync.dma_start(out=bias_sb[:], in_=bias_m[:])
            for g in range(1, min(3, NGRP)):
                pre.append(issue_group(g))

            ones_sb = cm_sb[:, 0:1]
            strip = strip_pool.tile([128, STRIP_COLS], F32)

            e_last = None
            for g in range(NGRP):
                cpd = PLAN[g]
                w_tile = pre[g] if g < len(pre) else issue_group(g)
                m = cpd * NBLK * B
                ps = pspool.tile([128, m], F32, tag="ps",
                                 padded_shape=[128, max(PLAN) * NBLK * B])
                for ci in range(cpd):
                    for blk in range(NBLK):
                        sub = ci * NBLK + blk
                        half = ps[:, sub * B:(sub + 1) * B]
                        for k in range(KT):
                            nc.tensor.matmul(
                                half,
                                lhsT=w_tile[:, (sub * KT + k) * 128:(sub * KT + k + 1) * 128],
                                rhs=p2_sb[:, k * B:(k + 1) * B],
                                start=(k == 0),
                                stop=(k == KT - 1 and not use_bias),
                            )
                        if use_bias:
                            ch = GSTART[g] + ci
                            nc.tensor.matmul(
                                half,
                                lhsT=bias_sb[0:1, ch * VSH + blk * 128:
                                             ch * VSH + (blk + 1) * 128],
                                rhs=bias_sb[1:2, 0:B],
                                start=False,
                                stop=True,
                            )
                if g < NFULL:
                    e_tile = epool.tile([128, m], BF16,
                                        padded_shape=[128, max(PLAN) * NBLK * B])
                    nc.scalar.activation(
                        e_tile[:], ps[:], mybir.ActivationFunctionType.Exp,
                        scale=1.0 / (MM_SCALE * MM_SCALE),
                    )
                    nc.tensor.matmul(
                        strip[0:m, g:g + 1], lhsT=e_tile[:], rhs=ones_sb,
                        start=True, stop=True,
                    )
                    nc.tensor.matmul(
                        strip[0:m, SGATHER + g * B:SGATHER + (g + 1) * B],
                        lhsT=e_tile[:], rhs=cm_sb[:, 1 + g * B:1 + (g + 1) * B],
                        start=True, stop=True,
                    )
                    if g == NFULL - 1:
                        nc.vector.tensor_copy(out=strip_sb[:], in_=strip[:])
                        if PLAIN_OUT:
                            nc.sync.dma_start(
                                out=strip_out[:],
                                in_=strip_sb[:].rearrange("p (a b n) -> p a b n",
                                                          a=1, b=1))
                        else:
                            # emitted after the producer so Tile defers the
                            # RAW edge to the trigger; the prep itself has no
                            # sync deps and runs early on the idle Pool engine
                            nc.gpsimd.kv_writeback(
                                strip_out[:],
                                strip_sb[:].rearrange("p (a b n) -> p a b n",
                                                      a=1, b=1),
                                ctx_sb[:], prepare_only=True, sem=sem_strip,
                                queue_num=0,
                            )
                else:
                    # last group: export raw (scaled) logits; the host
                    # applies exp and the reductions for this one chunk
                    nc.vector.tensor_copy(out=l8_sb[:], in_=ps[:])
                    if PLAIN_OUT:
                        nc.sync.dma_start(
                            out=l8_out[:],
                            in_=l8_sb[:].rearrange("p (a b n) -> p a b n", a=1, b=1))
                    else:
                        nc.gpsimd.kv_writeback(
                            l8_out[:],
                            l8_sb[:].rearrange("p (a b n) -> p a b n", a=1, b=1),
                            ctx_sb[:], prepare_only=True, sem=sem_l8, queue_num=1,
                        )
            if not PLAIN_OUT:
                # placeholder gate: _fixup_prepared_dmas rewrites this wait
                # to the data waits Tile attached to the preps (the DVE
                # copy ticks), so both triggers fire only once the export
                # source tiles are written
                nc.gpsimd.wait_ge(sem_gate, 0)
                nc.gpsimd.trigger_dma(count=None, queue_num=0)
                nc.gpsimd.trigger_dma(count=None, queue_num=1)
    nc.compile()
    if not PLAIN_OUT and os.environ.get("CPDIST_NOFIX", "0") != "1":
        _fixup_prepared_dmas(nc)
    return nc


def _fixup_prepared_dmas(nc):
    """Two post-compile repairs for the prepare_only+trigger flow:

    1. Tile's epilogue waits on the global DMASW{q} lane sem for each SWDGE
       prep, but leaves the prep's DMA-completion update pointing at the
       caller-supplied sem — rewrite on_update[0] to the lane sem.
    2. Tile's kv_writeback prep keeps a sync wait on the data producer,
       which puts the ~1us descriptor generation on the post-producer
       critical path. The descriptors only encode addresses, so drop the
       prep's non-Pool data waits; the data ordering is carried by the
       trigger's signals_writable dependency instead.
    """
    fn = nc.m.functions[0]
    insts = [i for b in fn.blocks for i in b.instructions]
    lane_sems = {}
    for ins in insts:
        si = ins.sync_info
        if si is None:
            continue
        for w in list(si.on_wait) + list(si.on_update):
            nm = w.ant_name or ""
            if nm.startswith("DMASW"):
                lane_sems[int(nm[5:].split("_")[0])] = (w.id, nm)
    data_waits = {}
    for ins in insts:
        if getattr(ins, "gen_mode", 0) != 1 or not hasattr(ins, "queue_num"):
            continue
        q = ins.queue_num
        si = ins.sync_info
        upd = list(si.on_update)
        sid, nm = lane_sems[q]
        upd[0] = mybir.SyncUpdate(
            sync_type=upd[0].sync_type, id=sid, ant_name=nm,
            update_mode=upd[0].update_mode, update_value=16,
        )
        si.on_update = upd
        # strip the data waits off the prep (desc-gen only encodes
        # addresses); collect them for the gate EventSemaphore instead
        kept = []
        for w in si.on_wait:
            if (w.ant_name or "").startswith("Pool"):
                kept.append(w)
            else:
                prev = data_waits.get(w.id)
                if prev is None or (w.wait_value or 0) > (prev.wait_value or 0):
                    data_waits[w.id] = w
        si.on_wait = kept
    assert data_waits, "no data waits found to move onto the trigger gate"
    for ins in insts:
        si = ins.sync_info
        if si is None or not si.on_wait:
            continue
        if any((w.ant_name or "") == "out_gate" for w in si.on_wait):
            si.on_wait = list(data_waits.values())
            break
    else:
        raise AssertionError("out_gate placeholder wait not found")


def _get_nc(use_bias=False):
    key = (tuple(PLAN), WBUFS, PSBUFS, use_bias)
    if key not in _cached:
        _cached[key] = _build_nc(use_bias)
    return _cached[key]


def _prep_core_inputs(W, bias_vec, points, ht):
    W4 = W.reshape(H, V, R, D)

    # moving operand: h^T tiled into KT k-tiles of [128, B]
    ht_t = np.ascontiguousarray(
        ht.reshape(KT, 128, B).transpose(1, 0, 2).reshape(128, KT * B)
    )
    pack2 = _to_mm(ht_t, MM_SCALE)

    # gather masks: group g covers chunks [GSTART[g], GSTART[g]+PLAN[g]); all
    # chunks in one group share the same horizon h = chunk//R. mask col for
    # (g, b) is one-hot at row (points[b, h] % VSH) % 128. Identical on every
    # core; non-owner entries are junk the host ignores.
    cmask = np.zeros((128, 1 + B * NFULL), mybir.dt.np(BF16))
    cmask[:, 0] = 1.0
    for g in range(NFULL):
        hg = GSTART[g] // R
        for b in range(B):
            row = (int(points[b, hg]) % VSH) % 128
            cmask[row, 1 + g * B + b] = 1.0

    use_bias = bool(np.any(bias_vec))
    in_maps = []
    for c in range(NCORES):
        sl = slice(c * VSH, (c + 1) * VSH)
        # (h, v, r, d) -> [p, (h, r, blk, kt, j)]: chunk-major per partition
        # so every group DMA is a contiguous per-partition slice
        s6 = W4[:, sl, :, :].reshape(H, NBLK, 128, R, KT, 128)
        slab = np.ascontiguousarray(s6.transpose(5, 0, 3, 1, 4, 2))
        slab = _to_mm(slab.reshape(128, NCHUNK * CCOLS), MM_SCALE)
        m = {"wt": slab, "pack2": pack2, "cmask": cmask}
        if use_bias:
            b3 = bias_vec.reshape(H, V, R)[:, sl, :]
            bm = np.zeros((2, NCHUNK * VSH), np.float32)
            bm[0] = np.ascontiguousarray(
                b3.transpose(0, 2, 1)).reshape(-1) * np.float32(MM_SCALE * MM_SCALE)
            bm[1, 0:B] = 1.0
            m["bias_m"] = bm
        in_maps.append(m)
    return in_maps, use_bias


def _build_fast(nc):
    """Cache a jitted executor for this nc so repeat kernel() calls skip
    retracing/recompiling (mirrors bass2jax.run_bass_via_pjrt)."""
    import jax
    from concourse import bass2jax
    from concourse.bass2jax import _bass_exec_p, partition_id_tensor
    from jax.experimental.shard_map import shard_map
    from jax.sharding import Mesh, PartitionSpec

    bass2jax.install_neuronx_cc_hook()
    partition_name = nc.partition_id_tensor.name if nc.partition_id_tensor else None
    in_names, out_names, out_avals, zero_outs = [], [], [], []
    for alloc in nc.m.functions[0].allocations:
        if not isinstance(alloc, mybir.MemoryLocationSet):
            continue
        name = alloc.memorylocations[0].name
        if alloc.kind == "ExternalInput":
            if name != partition_name:
                in_names.append(name)
        elif alloc.kind == "ExternalOutput":
            out_names.append(name)
            shape = tuple(alloc.tensor_shape)
            dtype = mybir.dt.np(alloc.dtype)
            out_avals.append(jax.core.ShapedArray(shape, dtype))
            zero_outs.append(np.zeros(shape, dtype))
    n_params = len(in_names)
    all_in = list(in_names) + list(out_names)
    if partition_name is not None:
        all_in.append(partition_name)

    def _body(*args):
        ops = list(args)
        if partition_name is not None:
            ops.append(partition_id_tensor())
        return tuple(
            _bass_exec_p.bind(
                *ops,
                out_avals=tuple(out_avals),
                in_names=tuple(all_in),
                out_names=tuple(out_names),
                lowering_input_output_aliases=(),
                sim_require_finite=True,
                sim_require_nnan=True,
                nc=nc,
            )
        )

    devices = jax.devices()[:NCORES]
    mesh = Mesh(np.asarray(devices), ("core",))
    spec = PartitionSpec("core")
    fn = jax.jit(
        shard_map(
            _body, mesh=mesh,
            in_specs=(spec,) * (n_params + len(out_names)),
            out_specs=(spec,) * len(out_names), check_rep=False,
        ),
        keep_unused=True,
    )
    _fast[id(nc)] = (fn, in_names, out_names, out_avals, zero_outs, mesh, spec)


def _run_cached(nc, in_maps):
    fn, in_names, out_names, out_avals, zero_outs, mesh, spec = _fast[id(nc)]
    concat_in = [
        np.concatenate([np.asarray(in_maps[c][nm]) for c in range(NCORES)], axis=0)
        for nm in in_names
    ]
    concat_zero = [
        np.zeros((NCORES * z.shape[0], *z.shape[1:]), z.dtype) for z in zero_outs
    ]
    outs = fn(*concat_in, *concat_zero)
    return [
        {
            nm: np.asarray(outs[i]).reshape(NCORES, *out_avals[i].shape)[c]
            for i, nm in enumerate(out_names)
        }
        for c in range(NCORES)
    ]


def _chunk_group(ch):
    for g in range(NGRP):
        if GSTART[g] <= ch < GSTART[g] + PLAN[g]:
            return g, ch - GSTART[g]
    raise AssertionError(ch)


def kernel(last_hidden_state, param_w, param_b, points):
    global _last_results
    from concourse.bass_utils import run_bass_kernel_spmd

    lhs = np.asarray(last_hidden_state, dtype=np.float32)
    W = np.ascontiguousarray(np.asarray(param_w, dtype=np.float32))
    bias_vec = np.asarray(param_b, dtype=np.float32)
    pts = np.asarray(points)

    ht = np.ascontiguousarray(lhs[:, -1, :].T)  # (D, B)
    in_maps, use_bias = _prep_core_inputs(W, bias_vec, pts, ht)

    nc = _get_nc(use_bias=use_bias)
    if id(nc) in _fast:
        results = _run_cached(nc, in_maps)
    else:
        res = run_bass_kernel_spmd(nc, in_maps, core_ids=list(range(NCORES)))
        _last_results = res
        results = res.results
        _build_fast(nc)

    # host combine: vocab sums S[b, h, r] summed across cores + subtiles;
    # p_eval factors gathered from the owning core's strip / raw tile
    S = np.zeros((B, H, R), np.float64)
    e8s = []
    strips = []
    for r in results:
        strip = r["strip_out"].reshape(128, STRIP_COLS).astype(np.float64)
        l8 = r["l8_out"].reshape(128, LAST_CPD * NBLK * B).astype(np.float64)
        e8 = np.exp(l8 / (MM_SCALE * MM_SCALE))
        strips.append(strip)
        e8s.append(e8)
        for g in range(NFULL):
            cpd = PLAN[g]
            col = strip[0:cpd * NBLK * B, g].reshape(cpd, NBLK, B)
            for ci in range(cpd):
                ch = GSTART[g] + ci
                S[:, ch // R, ch % R] += col[ci].sum(axis=0)
        # last group from the raw exp tile
        eb = e8.reshape(128, LAST_CPD, NBLK, B)
        for ci in range(LAST_CPD):
            ch = GSTART[NGRP - 1] + ci
            S[:, ch // R, ch % R] += eb[:, ci].sum(axis=(0, 1))

    a = np.zeros((B, H, R), np.float64)
    for b in range(B):
        for h in range(H):
            v = int(pts[b, h])
            co, vl = v // VSH, v % VSH
            blk, row = vl // 128, vl % 128
            for r in range(R):
                ch = h * R + r
                g, ci = _chunk_group(ch)
                if g < NFULL:
                    a[b, h, r] = strips[co][(ci * NBLK + blk) * B + b,
                                            SGATHER + g * B + b]
                else:
                    a[b, h, r] = e8s[co][row, (ci * NBLK + blk) * B + b]

    norm_const = (S[:, 0, :] * S[:, 1, :]).sum(axis=1)
    p_eval = (a[:, 0, :] * a[:, 1, :]).sum(axis=1)
    return p_eval.astype(np.float32), norm_const.astype(np.float32)


# revision 43
# speedup vs baseline: 1.0191x; 1.0191x over previous
"""Trainium2 Bass kernel for nn_CPDist.

Math: with a = exp(h_last @ W.T + b).reshape(B, H, V, R), the reference
computes p_tilde[b,i,j] = sum_r a[b,0,i,r]*a[b,1,j,r], then
  p_eval[b]     = p_tilde[b, p0, p1]
  norm_const[b] = sum_ij p_tilde[b,i,j]
Both factorize over the rank dim, so the (B,V,V) slab is never needed:
  norm_const[b] = sum_r (sum_i a[b,0,i,r]) * (sum_j a[b,1,j,r])
  p_eval[b]     = sum_r a[b,0,p0,r] * a[b,1,p1,r]
The dominant cost is the (B=8, D=1024) x (D, V*R*H=131072) matmul + exp —
HBM-bound on streaming the fp8 weight slab (16.8 MB per core).

Sharding: vocab dim V split across 8 cores (512 vocab rows each, for both
horizon slots).

Orientation: weights are the STATIONARY matmul operand (lhsT tiles of
[K=128, M=128 vocab]) and the hidden state h^T is the moving operand
([K=128, N=8]), so each matmul produces a [128 vocab, 8 batch] psum tile.
exp runs on [128, up to 128] tiles (full partition usage), vocab sums are
one ones-matmul per group (partition-dim reduction on the PE), and the
p_eval gather is one mask-matmul per group against host-built one-hot
columns. The last (1-chunk) group's exp tile is exported raw and reduced
on the host so the post-DMA tail stays minimal.
"""

import os

import numpy as np

import concourse.bacc as bacc
import concourse.bass as bass
import concourse.mybir as mybir
import concourse.tile as tile

B, T, D = 8, 128, 1024
V, R, H = 4096, 16, 2
NCORES = 8
VSH = V // NCORES            # vocab rows per core (512)
KT = D // 128                # 8 contraction tiles
NCHUNK = H * R               # 32 chunks of 512 vocab columns per core
NBLK = VSH // 128            # 4 vocab subtiles of 128 per chunk
CCOLS = NBLK * KT * 128      # 4096 weight-slab columns per chunk

F32 = mybir.dt.float32
BF16 = mybir.dt.bfloat16

MM_DTYPE = mybir.dt.float8e4
MM_SCALE = 1024.0

# chunks per weight DMA group; the final groups are small so the post-DMA
# drain only has one chunk of compute left
PLAN = [int(x) for x in os.environ.get("CPDIST_PLAN", "4,4,4,4,4,4,4,3,1").split(",")]
assert sum(PLAN) == NCHUNK
NGRP = len(PLAN)
GSTART = [sum(PLAN[:i]) for i in range(NGRP)]
NFULL = NGRP - 1             # groups that compute strip sums on device
LAST_CPD = PLAN[-1]

WBUFS = int(os.environ.get("CPDIST_WBUFS", "4"))
PSBUFS = int(os.environ.get("CPDIST_PSBUFS", "4"))
# plain HWDGE dma_start for the exports (deterministic); "0" enables the
# experimental triggered kv_writeback path, which is faster in the timeline
# model but races in the executor (Tile's prepared-DMA gating is unreliable
# when the prep precedes the producer)
PLAIN_OUT = os.environ.get("CPDIST_PLAIN_OUT", "1") == "1"

# strip column layout: cols [0, NFULL) = per-group vocab sums,
# cols [NFULL, NFULL + 8*NFULL) = per-group gather outputs
SGATHER = NFULL
STRIP_COLS = NFULL + B * NFULL

_cached = {}
_fast = {}
_last_results = None


def _to_mm(x, scale=1.0):
    x = np.ascontiguousarray(x, dtype=np.float32)
    if scale != 1.0:
        x = x * np.float32(scale)
    return x.astype(mybir.dt.np(MM_DTYPE))


def _build_nc(use_bias=False):
    nc = bacc.Bacc("TRN2", target_bir_lowering=False, num_swdge_queues=2)
    wt = nc.dram_tensor("wt", (128, NCHUNK * CCOLS), MM_DTYPE, kind="ExternalInput")
    pack2 = nc.dram_tensor("pack2", (128, KT * B), MM_DTYPE, kind="ExternalInput")
    # col 0: ones column for the vocab-sum matmuls; cols 1..1+8*NFULL:
    # one-hot gather masks (8 batch points per full group)
    cmask = nc.dram_tensor("cmask", (128, 1 + B * NFULL), BF16, kind="ExternalInput")
    if use_bias:
        bias_m = nc.dram_tensor("bias_m", (2, NCHUNK * VSH), F32, kind="ExternalInput")
    LCOLS = LAST_CPD * NBLK * B
    # exports are kv_writeback-shaped: [batch=1, d_head_inner=128, d_head_outer=1, n]
    strip_out = nc.dram_tensor("strip_out", (1, 128, 1, STRIP_COLS), BF16,
                               kind="ExternalOutput")
    l8_out = nc.dram_tensor("l8_out", (1, 128, 1, LCOLS), BF16, kind="ExternalOutput")

    with tile.TileContext(nc) as tc:
        with (
            tc.tile_pool(name="consts", bufs=1) as consts,
            tc.tile_pool(name="wpool", bufs=WBUFS) as wpool,
            tc.tile_pool(name="pspool", bufs=PSBUFS, space="PSUM") as pspool,
            tc.tile_pool(name="strip_pool", bufs=1, space="PSUM") as strip_pool,
            tc.tile_pool(name="epool", bufs=3) as epool,
            tc.tile_pool(name="spool", bufs=2) as spool,
        ):
            # descriptors for the two triggered output writebacks are
            # prepared up-front on the Pool engine; only the cheap
            # trigger_dma sits on the end-of-kernel critical path
            ctx_sb = consts.tile([128, 1], mybir.dt.int32, name="ctx_idx")
            nc.gpsimd.memset(ctx_sb[:], 0)
            strip_sb = spool.tile([128, STRIP_COLS], BF16, name="strip_sb")
            l8_sb = spool.tile([128, LCOLS], BF16, name="l8_sb")
            if not PLAIN_OUT:
                sem_strip = nc.alloc_semaphore("swdge_strip")
                sem_l8 = nc.alloc_semaphore("swdge_l8")
                # preps emitted up-front so their ~1us descriptor generation
                # runs on the idle Pool engine at kernel start; the data
                # ordering is carried by the gate wait in front of the
                # triggers (rewritten post-compile to the DVE copy ticks)
                nc.gpsimd.kv_writeback(
                    strip_out[:],
                    strip_sb[:].rearrange("p (a b n) -> p a b n", a=1, b=1),
                    ctx_sb[:], prepare_only=True, sem=sem_strip, queue_num=0,
                )
                nc.gpsimd.kv_writeback(
                    l8_out[:],
                    l8_sb[:].rearrange("p (a b n) -> p a b n", a=1, b=1),
                    ctx_sb[:], prepare_only=True, sem=sem_l8, queue_num=1,
                )

            def issue_group(g):
                cpd = PLAN[g]
                w_tile = wpool.tile([128, cpd * CCOLS], MM_DTYPE,
                                    padded_shape=[128, max(PLAN) * CCOLS],
                                    name=f"w_tile_{g}", tag="w_tile")
                nc.sync.dma_start(
                    out=w_tile[:],
                    in_=wt[:, GSTART[g] * CCOLS:(GSTART[g] + cpd) * CCOLS],
                )
                return w_tile

            # first weight group goes ahead of everything so the DMA engines
            # start on the bulk stream as early as possible
            pre = [issue_group(0)]
            p2_sb = consts.tile([128, KT * B], MM_DTYPE)
            nc.sync.dma_start(out=p2_sb[:], in_=pack2[:])
            cm_sb = consts.tile([128, 1 + B * NFULL], BF16)
            nc.sync.dma_start(out=cm_sb[:], in_=cmask[:])
            if use_bias:
                bias_sb = consts.tile([2, NCHUNK * VSH], F32)
                nc.sync.dma_start(out=bias_sb[:], in_=bias_m[:])
            for g in range(1, min(3, NGRP)):
                pre.append(issue_group(g))

            ones_sb = cm_sb[:, 0:1]
            strip = strip_pool.tile([128, STRIP_COLS], F32)

            e_last = None
            for g in range(NGRP):
                cpd = PLAN[g]
                w_tile = pre[g] if g < len(pre) else issue_group(g)
                m = cpd * NBLK * B
                ps = pspool.tile([128, m], F32, tag="ps",
                                 padded_shape=[128, max(PLAN) * NBLK * B])
                for ci in range(cpd):
                    for blk in range(NBLK):
                        sub = ci * NBLK + blk
                        half = ps[:, sub * B:(sub + 1) * B]
                        for k in range(KT):
                            nc.tensor.matmul(
                                half,
                                lhsT=w_tile[:, (sub * KT + k) * 128:(sub * KT + k + 1) * 128],
                                rhs=p2_sb[:, k * B:(k + 1) * B],
                                start=(k == 0),
                                stop=(k == KT - 1 and not use_bias),
                            )
                        if use_bias:
                            ch = GSTART[g] + ci
                            nc.tensor.matmul(
                                half,
                                lhsT=bias_sb[0:1, ch * VSH + blk * 128:
                                             ch * VSH + (blk + 1) * 128],
                                rhs=bias_sb[1:2, 0:B],
                                start=False,
                                stop=True,
                            )
                if g < NFULL:
                    e_tile = epool.tile([128, m], BF16,
                                        padded_shape=[128, max(PLAN) * NBLK * B])
                    nc.scalar.activation(
                        e_tile[:], ps[:], mybir.ActivationFunctionType.Exp,
                        scale=1.0 / (MM_SCALE * MM_SCALE),
                    )
                    nc.tensor.matmul(
                        strip[0:m, g:g + 1], lhsT=e_tile[:], rhs=ones_sb,
                        start=True, stop=True,
                    )
                    nc.tensor.matmul(
                        strip[0:m, SGATHER + g * B:SGATHER + (g + 1) * B],
                        lhsT=e_tile[:], rhs=cm_sb[:, 1 + g * B:1 + (g + 1) * B],
                        start=True, stop=True,
                    )
                    if g == NFULL - 1:
                        nc.vector.tensor_copy(out=strip_sb[:], in_=strip[:])
                        if PLAIN_OUT:
                            # ACT ring: overlaps this export's HWDGE/DGE
                            # chain with the l8 export's on the SP ring
                            nc.scalar.dma_start(
                                out=strip_out[:],
                                in_=strip_sb[:].rearrange("p (a b n) -> p a b n",
                                                          a=1, b=1))
                else:
                    # last group: export raw (scaled) logits; the host
                    # applies exp and the reductions for this one chunk
                    nc.vector.tensor_copy(out=l8_sb[:], in_=ps[:])
                    if PLAIN_OUT:
                        nc.sync.dma_start(
                            out=l8_out[:],
                            in_=l8_sb[:].rearrange("p (a b n) -> p a b n", a=1, b=1))
            if not PLAIN_OUT:
                # data gating comes from Tile's own IncSwdgeSem epilogue
                # (waits the DVE copy ticks) which precedes these on Pool
                nc.gpsimd.trigger_dma(count=None, queue_num=0)
                nc.gpsimd.trigger_dma(count=None, queue_num=1)
    nc.compile()
    if not PLAIN_OUT and os.environ.get("CPDIST_NOFIX", "0") != "1":
        _fixup_prepared_dmas(nc)
    _spread_const_memsets(nc)
    return nc


def _spread_const_memsets(nc):
    """The Bass preamble memsets four const-AP tensors serially on the Pool
    engine (~95ns Q7 launch each), delaying the initial all-engine barrier
    and hence the first weight DMA. Only const-float32-0.0 is ever read here
    (the implicit activation bias); drop the other three memsets."""
    if os.environ.get("CPDIST_NOSPREAD", "0") == "1":
        return
    fn = nc.m.functions[0]
    blk = fn.blocks[0]
    il = blk.instructions
    keep = [
        ins for ins in il
        if not (
            type(ins).__name__ == "InstMemset"
            and (getattr(ins.outs[0], "memref", "") or "").startswith("const-")
            and "float32-0.0" not in ins.outs[0].memref
        )
    ]
    removed = len(il) - len(keep)
    assert removed == 3, removed
    il[:] = keep


def _fixup_prepared_dmas(nc):
    """Two post-compile repairs for the prepare_only+trigger flow:

    1. Tile's epilogue waits on the global DMASW{q} lane sem for each SWDGE
       prep, but leaves the prep's DMA-completion update pointing at the
       caller-supplied sem — rewrite on_update[0] to the lane sem.
    2. Tile's kv_writeback prep keeps a sync wait on the data producer,
       which puts the ~1us descriptor generation on the post-producer
       critical path. The descriptors only encode addresses, so drop the
       prep's non-Pool data waits; the data ordering is carried by the
       trigger's signals_writable dependency instead.
    """
    fn = nc.m.functions[0]
    insts = [i for b in fn.blocks for i in b.instructions]
    lane_sems = {}
    for ins in insts:
        si = ins.sync_info
        if si is None:
            continue
        for w in list(si.on_wait) + list(si.on_update):
            nm = w.ant_name or ""
            if nm.startswith("DMASW"):
                lane_sems[int(nm[5:].split("_")[0])] = (w.id, nm)
    for ins in insts:
        if getattr(ins, "gen_mode", 0) != 1 or not hasattr(ins, "queue_num"):
            continue
        q = ins.queue_num
        si = ins.sync_info
        upd = list(si.on_update)
        sid, nm = lane_sems[q]
        upd[0] = mybir.SyncUpdate(
            sync_type=upd[0].sync_type, id=sid, ant_name=nm,
            update_mode=upd[0].update_mode, update_value=16,
        )
        si.on_update = upd
        # desc-gen only encodes addresses — it must not wait on data
        si.on_wait = [w for w in si.on_wait if (w.ant_name or "").startswith("Pool")]
    # Tile guards the DVE copies with a WAR wait on the deferred prepared-DMA
    # read (wait DMASW >= 16 before overwriting the source tile). The source
    # tiles are written exactly once here, so the guard is vacuous — and with
    # the prep emitted first it deadlocks against Tile's own IncSwdgeSem data
    # gate. Clear those mid-stream waits (the SP epilogue drains keep theirs).
    ncleared = 0
    for ins in insts:
        if type(ins).__name__ != "InstEventSemaphore":
            continue
        if ins.engine != mybir.EngineType.DVE:
            continue
        si = ins.sync_info
        if si is None or not si.on_wait:
            continue
        if all((w.ant_name or "").startswith("DMASW") for w in si.on_wait):
            si.on_wait = []
            ncleared += 1
    assert ncleared >= 1, "expected to clear the DVE WAR guard waits"


def _get_nc(use_bias=False):
    key = (tuple(PLAN), WBUFS, PSBUFS, use_bias)
    if key not in _cached:
        _cached[key] = _build_nc(use_bias)
    return _cached[key]


def _prep_core_inputs(W, bias_vec, points, ht):
    W4 = W.reshape(H, V, R, D)

    # moving operand: h^T tiled into KT k-tiles of [128, B]
    ht_t = np.ascontiguousarray(
        ht.reshape(KT, 128, B).transpose(1, 0, 2).reshape(128, KT * B)
    )
    pack2 = _to_mm(ht_t, MM_SCALE)

    # gather masks: group g covers chunks [GSTART[g], GSTART[g]+PLAN[g]); all
    # chunks in one group share the same horizon h = chunk//R. mask col for
    # (g, b) is one-hot at row (points[b, h] % VSH) % 128. Identical on every
    # core; non-owner entries are junk the host ignores.
    cmask = np.zeros((128, 1 + B * NFULL), mybir.dt.np(BF16))
    cmask[:, 0] = 1.0
    for g in range(NFULL):
        hg = GSTART[g] // R
        for b in range(B):
            row = (int(points[b, hg]) % VSH) % 128
            cmask[row, 1 + g * B + b] = 1.0

    use_bias = bool(np.any(bias_vec))
    in_maps = []
    for c in range(NCORES):
        sl = slice(c * VSH, (c + 1) * VSH)
        # (h, v, r, d) -> [p, (h, r, blk, kt, j)]: chunk-major per partition
        # so every group DMA is a contiguous per-partition slice
        s6 = W4[:, sl, :, :].reshape(H, NBLK, 128, R, KT, 128)
        slab = np.ascontiguousarray(s6.transpose(5, 0, 3, 1, 4, 2))
        slab = _to_mm(slab.reshape(128, NCHUNK * CCOLS), MM_SCALE)
        m = {"wt": slab, "pack2": pack2, "cmask": cmask}
        if use_bias:
            b3 = bias_vec.reshape(H, V, R)[:, sl, :]
            bm = np.zeros((2, NCHUNK * VSH), np.float32)
            bm[0] = np.ascontiguousarray(
                b3.transpose(0, 2, 1)).reshape(-1) * np.float32(MM_SCALE * MM_SCALE)
            bm[1, 0:B] = 1.0
            m["bias_m"] = bm
        in_maps.append(m)
    return in_maps, use_bias


def _build_fast(nc):
    """Cache a jitted executor for this nc so repeat kernel() calls skip
    retracing/recompiling (mirrors bass2jax.run_bass_via_pjrt)."""
    import jax
    from concourse import bass2jax
    from concourse.bass2jax import _bass_exec_p, partition_id_tensor
    from jax.experimental.shard_map import shard_map
    from jax.sharding import Mesh, PartitionSpec

    bass2jax.install_neuronx_cc_hook()
    partition_name = nc.partition_id_tensor.name if nc.partition_id_tensor else None
    in_names, out_names, out_avals, zero_outs = [], [], [], []
    for alloc in nc.m.functions[0].allocations:
        if not isinstance(alloc, mybir.MemoryLocationSet):
            continue
        name = alloc.memorylocations[0].name
        if alloc.kind == "ExternalInput":
            if name != partition_name:
                in_names.append(name)
        elif alloc.kind == "ExternalOutput":
            out_names.append(name)
            shape = tuple(alloc.tensor_shape)
            dtype = mybir.dt.np(alloc.dtype)
            out_avals.append(jax.core.ShapedArray(shape, dtype))
            zero_outs.append(np.zeros(shape, dtype))
    n_params = len(in_names)
    all_in = list(in_names) + list(out_names)
    if partition_name is not None:
        all_in.append(partition_name)

    def _body(*args):
        ops = list(args)
        if partition_name is not None:
            ops.append(partition_id_tensor())
        return tuple(
            _bass_exec_p.bind(
                *ops,
                out_avals=tuple(out_avals),
                in_names=tuple(all_in),
                out_names=tuple(out_names),
                lowering_input_output_aliases=(),
                sim_require_finite=True,
                sim_require_nnan=True,
                nc=nc,
            )
        )

    devices = jax.devices()[:NCORES]
    mesh = Mesh(np.asarray(devices), ("core",))
    spec = PartitionSpec("core")
    fn = jax.jit(
        shard_map(
            _body, mesh=mesh,
            in_specs=(spec,) * (n_params + len(out_names)),
            out_specs=(spec,) * len(out_names), check_rep=False,
        ),
        keep_unused=True,
    )
    _fast[id(nc)] = (fn, in_names, out_names, out_avals, zero_outs, mesh, spec)


def _run_cached(nc, in_maps):
    fn, in_names, out_names, out_avals, zero_outs, mesh, spec = _fast[id(nc)]
    concat_in = [
        np.concatenate([np.asarray(in_maps[c][nm]) for c in range(NCORES)], axis=0)
        for nm in in_names
    ]
    concat_zero = [
        np.zeros((NCORES * z.shape[0], *z.shape[1:]), z.dtype) for z in zero_outs
    ]
    outs = fn(*concat_in, *concat_zero)
    return [
        {
            nm: np.asarray(outs[i]).reshape(NCORES, *out_avals[i].shape)[c]
            for i, nm in enumerate(out_names)
        }
        for c in range(NCORES)
    ]


def _chunk_group(ch):
    for g in range(NGRP):
        if GSTART[g] <= ch < GSTART[g] + PLAN[g]:
            return g, ch - GSTART[g]
    raise AssertionError(ch)


def kernel(last_hidden_state, param_w, param_b, points):
    global _last_results
    from concourse.bass_utils import run_bass_kernel_spmd

    lhs = np.asarray(last_hidden_state, dtype=np.float32)
    W = np.ascontiguousarray(np.asarray(param_w, dtype=np.float32))
    bias_vec = np.asarray(param_b, dtype=np.float32)
    pts = np.asarray(points)

    ht = np.ascontiguousarray(lhs[:, -1, :].T)  # (D, B)
    in_maps, use_bias = _prep_core_inputs(W, bias_vec, pts, ht)

    nc = _get_nc(use_bias=use_bias)
    if id(nc) in _fast:
        results = _run_cached(nc, in_maps)
    else:
        res = run_bass_kernel_spmd(nc, in_maps, core_ids=list(range(NCORES)))
        _last_results = res
        results = res.results
        _build_fast(nc)

    # host combine: vocab sums S[b, h, r] summed across cores + subtiles;
    # p_eval factors gathered from the owning core's strip / raw tile
    S = np.zeros((B, H, R), np.float64)
    e8s = []
    strips = []
    for r in results:
        strip = r["strip_out"].reshape(128, STRIP_COLS).astype(np.float64)
        l8 = r["l8_out"].reshape(128, LAST_CPD * NBLK * B).astype(np.float64)
        e8 = np.exp(l8 / (MM_SCALE * MM_SCALE))
        strips.append(strip)
        e8s.append(e8)
        for g in range(NFULL):
            cpd = PLAN[g]
            col = strip[0:cpd * NBLK * B, g].reshape(cpd, NBLK, B)
            for ci in range(cpd):
                ch = GSTART[g] + ci
                S[:, ch // R, ch % R] += col[ci].sum(axis=0)
        # last group from the raw exp tile
        eb = e8.reshape(128, LAST_CPD, NBLK, B)
        for ci in range(LAST_CPD):
            ch = GSTART[NGRP - 1] + ci
            S[:, ch // R, ch % R] += eb[:, ci].sum(axis=(0, 1))

    a = np.zeros((B, H, R), np.float64)
    for b in range(B):
        for h in range(H):
            v = int(pts[b, h])
            co, vl = v // VSH, v % VSH
            blk, row = vl // 128, vl % 128
            for r in range(R):
                ch = h * R + r
                g, ci = _chunk_group(ch)
                if g < NFULL:
                    a[b, h, r] = strips[co][(ci * NBLK + blk) * B + b,
                                            SGATHER + g * B + b]
                else:
                    a[b, h, r] = e8s[co][row, (ci * NBLK + blk) * B + b]

    norm_const = (S[:, 0, :] * S[:, 1, :]).sum(axis=1)
    p_eval = (a[:, 0, :] * a[:, 1, :]).sum(axis=1)
    return p_eval.astype(np.float32), norm_const.astype(np.float32)
